# revision 1
# baseline (speedup 1.0000x reference)
"""ListMLE loss kernel for Trainium2 (8 NeuronCores, Bass/Tile).

loss = mean(logcumsumexp(outputs[t, labels[t]], axis=1) - outputs)

Strategy (per core, rows sharded 1024/core):
  - 8 row-tiles of [128, 4096]: exp on ACT, per-row gather via GPSIMD
    ap_gather (per-core index lists; subcall s of 16 covers rows p≡s mod 16,
    host pre-wraps label slices so each Q7 core gets the right row's labels),
    inclusive cumsum via DVE tensor_tensor_scan, Ln+accumulate on ACT.
  - Garbage partitions from the shared-index gather are masked out at the
    end with a host-supplied {0,1} mask before the final reduction.
  - Per-core partial = (sum ln scores - sum outputs) / (B*N); host sums the
    8 partials (the all-reduce of the sharding hint).
"""

import numpy as np

import concourse.bacc as bacc
import concourse.mybir as mybir
import concourse.tile as tile
from concourse.bass_utils import run_bass_kernel_spmd

B, N = 8192, 4096
N_CORES = 8
ROWS = B // N_CORES      # 1024 rows per core
TILES = ROWS // 128      # 8
SUB = 16                 # gather subcalls per tile (one per partition-mod-16)
NW = N // 16             # wrapped index columns

_NC = None


def _build():
    nc = bacc.Bacc("TRN2", target_bir_lowering=False, debug=False,
                   num_devices=N_CORES)
    O = nc.dram_tensor("outputs", [ROWS, N], mybir.dt.float32,
                       kind="ExternalInput").ap()
    LW = nc.dram_tensor("lblw", [TILES * SUB * 128, NW], mybir.dt.int16,
                        kind="ExternalInput").ap()
    MK = nc.dram_tensor("mask", [128, SUB], mybir.dt.float32,
                        kind="ExternalInput").ap()
    OUT = nc.dram_tensor("out", [1, 1], mybir.dt.float32,
                         kind="ExternalOutput").ap()
    f32 = mybir.dt.float32
    add = mybir.AluOpType.add

    with tile.TileContext(nc) as tc:
        with tc.tile_pool(name="main", bufs=2) as pool, \
             tc.tile_pool(name="small", bufs=1) as spool:
            acc_sc = spool.tile([128, SUB], f32, tag="acc_sc")
            acc_o = spool.tile([128, 1], f32, tag="acc_o")
            mask = spool.tile([128, SUB], f32, tag="mask")
            nc.vector.memset(acc_sc[:], 0.0)
            nc.vector.memset(acc_o[:], 0.0)
            nc.sync.dma_start(out=mask[:], in_=MK[:])
            for t in range(TILES):
                o = pool.tile([128, N], f32, tag="o")
                nc.sync.dma_start(out=o[:], in_=O[128 * t:128 * (t + 1), :])
                e = pool.tile([128, N], f32, tag="e")
                nc.scalar.activation(e[:], o[:],
                                     mybir.ActivationFunctionType.Exp)
                osum = pool.tile([128, 1], f32, tag="osum")
                nc.vector.tensor_reduce(osum[:], o[:],
                                        axis=mybir.AxisListType.X, op=add)
                nc.vector.tensor_tensor(out=acc_o[:], in0=acc_o[:],
                                        in1=osum[:], op=add)
                for s in range(SUB):
                    idx = pool.tile([128, NW], mybir.dt.int16, tag="idx")
                    base = (t * SUB + s) * 128
                    nc.sync.dma_start(out=idx[:], in_=LW[base:base + 128, :])
                    g = pool.tile([128, N], f32, tag="g")
                    nc.gpsimd.ap_gather(g[:], e[:], idx[:], channels=128,
                                        num_elems=N, d=1, num_idxs=N)
                    sc = pool.tile([128, N], f32, tag="sc")
                    nc.vector.tensor_tensor_scan(sc[:], g[:], g[:], 0.0,
                                                 add, mybir.AluOpType.bypass)
                    lnt = pool.tile([128, N], f32, tag="lnt")
                    lnacc = pool.tile([128, 1], f32, tag="lnacc")
                    nc.scalar.activation(lnt[:], sc[:],
                                         mybir.ActivationFunctionType.Ln,
                                         accum_out=lnacc[:])
                    nc.vector.tensor_tensor(out=acc_sc[:, s:s + 1],
                                            in0=acc_sc[:, s:s + 1],
                                            in1=lnacc[:], op=add)
            mrow = spool.tile([128, SUB], f32, tag="mrow")
            nc.vector.tensor_tensor(out=mrow[:], in0=acc_sc[:], in1=mask[:],
                                    op=mybir.AluOpType.mult)
            mred = spool.tile([128, 1], f32, tag="mred")
            nc.vector.tensor_reduce(mred[:], mrow[:],
                                    axis=mybir.AxisListType.X, op=add)
            comb = spool.tile([128, 1], f32, tag="comb")
            nc.vector.tensor_tensor(out=comb[:], in0=mred[:], in1=acc_o[:],
                                    op=mybir.AluOpType.subtract)
            tot = spool.tile([1, 1], f32, tag="tot")
            nc.gpsimd.tensor_reduce(tot[:], comb[:],
                                    axis=mybir.AxisListType.C, op=add)
            res = spool.tile([1, 1], f32, tag="res")
            nc.scalar.mul(res[:], tot[:], 1.0 / (B * N))
            nc.sync.dma_start(out=OUT[:], in_=res[:])
    nc.compile()
    return nc


def _get_nc():
    global _NC
    if _NC is None:
        _NC = _build()
    return _NC


def _prep_inputs(outputs, labels):
    outputs = np.ascontiguousarray(np.asarray(outputs), dtype=np.float32)
    lab16 = np.asarray(labels).astype(np.int16)  # values in [0, 4096)
    mask = (np.arange(128)[:, None] % 16 == np.arange(SUB)[None, :]) \
        .astype(np.float32)
    in_maps = []
    for c in range(N_CORES):
        Oc = outputs[c * ROWS:(c + 1) * ROWS]
        Lc = lab16[c * ROWS:(c + 1) * ROWS]
        # lw[(t*16+s)*128 + 16c + p', i'] = Lc[128t + 16c + s, 16i' + p']
        L5 = Lc.reshape(TILES, 8, 16, NW, 16)          # [t, cg, s, i', p']
        lw = np.ascontiguousarray(
            L5.transpose(0, 2, 1, 4, 3).reshape(TILES * SUB * 128, NW))
        in_maps.append({"outputs": Oc, "lblw": lw, "mask": mask})
    return in_maps


def kernel(outputs, labels):
    nc = _get_nc()
    in_maps = _prep_inputs(outputs, labels)
    res = run_bass_kernel_spmd(nc, in_maps, core_ids=list(range(N_CORES)))
    total = sum(float(r["out"][0, 0]) for r in res.results)
    return np.float32(total)

